# revision 28
# baseline (speedup 1.0000x reference)
"""Trainium2 Bass kernel for nn_MeanAligning (VQ codebook mean-aligning loss).

Math (see reference):
    count[k] = sum_nhw code[nhw, k]
    num[k,c] = sum_nhw code[nhw, k] * quantized[nhw, c]
    mean     = num / count (count==0 -> mean 0)
    loss     = sum_{k: count>0} ||codebook[k] - mean[k]||^2 / (n_valid * C)

Sharding: K-parallel over the 4096 codebook entries — each of the 8 cores
gets a contiguous 512-column slice of `code` and ALL positions, so each
core owns the *complete* count/num for its K-shard. Only a [1,2] partial
(sum_sq, 32*n_valid) crosses cores at the end.

Device pipeline per core:
  - `code` is staged host-side as fp8e4 (one-hot 0/1 values are exact in
    fp8e4 — a lossless relayout) and `quant|ones` as bf16.
  - 128 PSUM-accumulated matmuls: lhsT = [quant|ones] tile [128pos, 33]
    (bf16), rhs = code tile [128pos, 512k] (fp8) -> psum [33, 512] f32
    holding (num^T ; count) — exact count, fp32-accumulated num.
  - Broadcast count across the 32 C partitions with a 1-partition matmul,
    masked squared diff, fused square+reduce, PE column-reduce -> [1,2].
  - Cross-core combine of the [1,2] partial (AllReduce or AllGather+sum),
    final scalar math, output.
"""

import os
import sys

import numpy as np

for _p in (
    "/opt/trn_rl_repo",
    "/root/.axon_site",
    "/root/.axon_site/_ro/trn_rl_repo",
):
    if os.path.isdir(_p) and _p not in sys.path:
        sys.path.append(_p)

import concourse.bass as bass  # noqa: E402
import concourse.mybir as mybir  # noqa: E402
import concourse.tile as tile  # noqa: E402
from concourse import bacc, bass_utils  # noqa: E402
from concourse.bass import ts  # noqa: E402

F32 = mybir.dt.float32
BF16 = mybir.dt.bfloat16
FP8 = (
    mybir.dt.float8e4
    if os.environ.get("MEAN_ALIGN_CODE_DT", "fp8") == "fp8"
    else mybir.dt.bfloat16
)
AOT = mybir.AluOpType

# Problem shapes (hardcoded per contract).
N, H, W, C, K = 16, 32, 32, 32, 4096
NHW = N * H * W            # 16384 positions
NCORES = 8
KS = K // NCORES           # 512 codebook entries per core
P = 128                    # partitions
S = NHW // P               # 128 position-tiles
GB = 16                    # position-tiles per DMA batch
NB = S // GB               # 8 DMA batches
CODE_ROWS = NB * P         # 1024
CODE_COLS = GB * KS        # 8192
C1 = C + 1                 # 33 = C + ones column

COLLECTIVE = os.environ.get("MEAN_ALIGN_COLLECTIVE", "ar")
RECIP = os.environ.get("MEAN_ALIGN_RECIP", "approx")
TCB_ENGINE = os.environ.get("MEAN_ALIGN_TCB", "gpsimd")
CODE_DT = os.environ.get("MEAN_ALIGN_CODE_DT", "fp8")
DOUBLE_ROW = os.environ.get("MEAN_ALIGN_DR", "0") == "1"
C1P = 48  # padded qo row length for DoubleRow (step%16==0 constraint)

_CACHE: dict = {}


def _build_nc(collective=COLLECTIVE):
    """Trace + compile the per-core Bass program (identical on all cores)."""
    nc = bacc.Bacc(
        "TRN2",
        target_bir_lowering=False,
        debug=False,
        enable_asserts=False,
        num_devices=NCORES,
    )

    # code_s[t*P + p, g*KS + k] = code[(t*GB+g)*P + p, k_shard_base + k]  (fp8)
    code_d = nc.dram_tensor("code_s", [CODE_ROWS, CODE_COLS], FP8, kind="ExternalInput").ap()
    if DOUBLE_ROW:
        # qo[p, (a*2+j)*48 + c] = [quant | ones | 0pad][(2a+j)*P + p, c]  (fp8)
        qo_d = nc.dram_tensor("qo", [P, S * C1P], FP8, kind="ExternalInput").ap()
    else:
        # qo[p, s*33 + c] = [quant | ones][s*P + p, c]  (bf16)
        qo_d = nc.dram_tensor("qo", [P, S * C1], BF16, kind="ExternalInput").ap()
    # cbT[c, k] = codebook[k_shard_base + k, c]  (f32)
    cb_d = nc.dram_tensor("cbt", [C, KS], F32, kind="ExternalInput").ap()
    out_shape = [1, 2] if collective == "host" else [1, 1]
    loss_d = nc.dram_tensor("loss", out_shape, F32, kind="ExternalOutput").ap()

    with tile.TileContext(nc) as tc:
        with (
            tc.tile_pool(name="consts", bufs=1) as consts,
            tc.tile_pool(name="codep", bufs=6) as codep,
            tc.tile_pool(name="work", bufs=1) as work,
            tc.tile_pool(name="acc_psum", bufs=1, space="PSUM") as acc_psum,
            tc.tile_pool(name="aux_psum", bufs=1, space="PSUM") as aux_psum,
            tc.tile_pool(name="dram", bufs=1, space="DRAM") as dram,
        ):
            # qo + cb go on the scalar HWDGE ring so they overlap the code
            # stream (sync ring). qo is split so early s-tiles unblock sooner.
            qo_cols = S * (C1P if DOUBLE_ROW else C1)
            qo_sb = consts.tile([P, qo_cols], FP8 if DOUBLE_ROW else BF16)
            q0 = qo_cols // 8
            nc.scalar.dma_start(qo_sb[:, 0:q0], qo_d[:, 0:q0])
            nc.scalar.dma_start(qo_sb[:, q0:], qo_d[:, q0:])
            cb_sb = consts.tile([C, KS], F32)
            nc.scalar.dma_start(cb_sb, cb_d)
            ones_sb = consts.tile([C1, C], F32)
            nc.vector.memset(ones_sb, 1.0)

            # ---- main streaming phase: num^T/count accumulation ----
            acc = acc_psum.tile([C1, KS], F32)  # rows 0..31 = num^T, row 32 = count
            for t in range(NB):
                ctile = codep.tile([P, CODE_COLS], FP8, tag="code")
                # alternate code batches across the two HWDGE rings so the
                # per-DMA fixed/receipt costs pipeline instead of serializing
                eng = nc.sync if t % 2 == 0 else nc.scalar
                if t == 0:
                    # split the first batch so matmul g=0 starts ASAP
                    q = CODE_COLS // 4
                    for i in range(4):
                        eng.dma_start(
                            ctile[:, i * q : (i + 1) * q],
                            code_d[ts(t, P), i * q : (i + 1) * q],
                        )
                else:
                    eng.dma_start(ctile, code_d[ts(t, P), :])
                if DOUBLE_ROW:
                    qo3 = qo_sb.rearrange("p (a j c) -> p a j c", j=2, c=C1P)
                    ct3 = ctile.rearrange("p (g k) -> p g k", k=KS)
                    for b in range(GB // 2):
                        a = (t * GB) // 2 + b
                        nc.tensor.matmul(
                            acc,
                            qo3[:, a, :, 0:C1],
                            ct3[:, 2 * b : 2 * b + 2, :],
                            start=(a == 0),
                            stop=(a == S // 2 - 1),
                            perf_mode=mybir.MatmulPerfMode.DoubleRow,
                        )
                else:
                    for g in range(GB):
                        s = t * GB + g
                        nc.tensor.matmul(
                            acc,
                            qo_sb[:, s * C1 : (s + 1) * C1],
                            ctile[:, g * KS : (g + 1) * KS],
                            start=(s == 0),
                            stop=(s == S - 1),
                        )

            # ---- broadcast count across the 32 C partitions via PE ----
            cnt0 = work.tile([1, KS], F32)
            nc.vector.tensor_copy(cnt0, acc[C : C1, :])  # psum row 32 -> partition 0
            c_ps = aux_psum.tile([C, KS], F32, tag="cbc")
            nc.tensor.matmul(c_ps, ones_sb[0:1, 0:C], cnt0, start=True, stop=True)

            # ---- per-(c,k) masked diff and partial sums (7-op DVE chain) ----
            safe = work.tile([C, KS], F32)
            nc.vector.tensor_scalar_max(safe, c_ps, 0.5)
            rcp = work.tile([C, KS], F32)
            if RECIP == "approx":
                nc.vector.reciprocal_approx_fast(rcp, safe)
            else:
                nc.vector.reciprocal(rcp, safe)
            valid = work.tile([C, KS], F32)
            nc.vector.tensor_scalar(valid, c_ps, 0.5, None, AOT.is_gt)

            mean = work.tile([C, KS], F32)
            nc.vector.tensor_mul(mean, acc[0:C, :], rcp)
            diff = work.tile([C, KS], F32)
            nc.vector.tensor_sub(diff, cb_sb, mean)
            sq = work.tile([C, KS], F32)
            nc.vector.tensor_mul(sq, diff, diff)
            msq = work.tile([C, KS], F32)
            nc.vector.tensor_mul(msq, sq, valid)

            stack = work.tile([C, 2], F32)
            nc.vector.reduce_sum(stack[:, 0:1], msq, axis=mybir.AxisListType.X)
            nc.vector.reduce_sum(stack[:, 1:2], valid, axis=mybir.AxisListType.X)

            fin_ps = aux_psum.tile([1, 2], F32, tag="fin")
            nc.tensor.matmul(fin_ps, ones_sb[0:C, 0:1], stack, start=True, stop=True)
            part = work.tile([1, 2], F32)
            nc.vector.tensor_copy(part, fin_ps)

            if collective == "host":
                nc.sync.dma_start(loss_d, part)
            else:
                _device_combine(
                    nc, tc, dram, work, aux_psum, ones_sb, part, loss_d, collective
                )

    nc.compile()
    return nc


def _device_combine(nc, tc, dram, work, aux_psum, ones_sb, part, loss_d, collective):
    # ---- cross-core combine of (sum_sq, 32*n_valid) ----
    if True:
        if True:
            cc_in = dram.tile([1, 2], F32)
            nc.sync.dma_start(cc_in, part)
            if collective == "ar":
                cc_out = dram.tile([1, 2], F32)
                nc.gpsimd.collective_compute(
                    "AllReduce",
                    AOT.add,
                    replica_groups=[list(range(NCORES))],
                    ins=[cc_in.opt()],
                    outs=[cc_out.opt()],
                )
                tot = work.tile([1, 2], F32)
                nc.sync.dma_start(tot, cc_out)
                tot_sq = tot[:, 0:1]
                tot_nv = tot[:, 1:2]
            else:  # "ag": AllGather + local sum
                cc_out = dram.tile([NCORES, 2], F32)
                nc.gpsimd.collective_compute(
                    "AllGather",
                    AOT.bypass,
                    replica_groups=[list(range(NCORES))],
                    ins=[cc_in.opt()],
                    outs=[cc_out.opt()],
                )
                gat = work.tile([NCORES, 2], F32)
                nc.sync.dma_start(gat, cc_out)
                tot_ps = aux_psum.tile([1, 2], F32, tag="tot")
                nc.tensor.matmul(
                    tot_ps, ones_sb[0:NCORES, 0:1], gat, start=True, stop=True
                )
                tot = work.tile([1, 2], F32)
                nc.vector.tensor_copy(tot, tot_ps)
                tot_sq = tot[:, 0:1]
                tot_nv = tot[:, 1:2]

            # loss = sum_sq / max(32*n_valid, 32)   (== sum_sq/(max(nv,1)*C))
            nv = work.tile([1, 1], F32)
            nc.vector.tensor_scalar_max(nv, tot_nv, float(C))
            rnv = work.tile([1, 1], F32)
            nc.vector.reciprocal(rnv, nv)
            res = work.tile([1, 1], F32)
            nc.vector.tensor_mul(res, tot_sq, rnv)
            nc.sync.dma_start(loss_d, res)


def _get_nc():
    if "nc" not in _CACHE:
        _CACHE["nc"] = _build_nc()
    return _CACHE["nc"]


def _make_in_maps(quantized, code, codebook):
    np_fp8 = mybir.dt.np(FP8)
    np_bf16 = mybir.dt.np(BF16)

    q2 = np.asarray(quantized, dtype=np.float32).reshape(NHW, C)
    code2 = np.asarray(code, dtype=np.float32).reshape(NHW, K)
    cb = np.asarray(codebook, dtype=np.float32)

    if DOUBLE_ROW:
        qo = np.zeros((NHW, C1P), np.float32)
        qo[:, 0:C] = q2
        qo[:, C] = 1.0
        qo_kc = np.ascontiguousarray(
            qo.reshape(S, P, C1P).swapaxes(0, 1)
        ).reshape(P, S * C1P).astype(np_fp8)
    else:
        qo = np.concatenate([q2, np.ones((NHW, 1), np.float32)], axis=1)  # [NHW, 33]
        # qo_kc[p, s*33+c] = qo[s*128+p, c]
        qo_kc = np.ascontiguousarray(
            qo.reshape(S, P, C1).swapaxes(0, 1)
        ).reshape(P, S * C1).astype(np_bf16)

    code8 = code2.astype(np_fp8)  # 0/1 values: exact
    in_maps = []
    for j in range(NCORES):
        ksl = slice(j * KS, (j + 1) * KS)
        # [NHW, KS] -> [NB, GB, P, KS] -> [NB, P, GB, KS] -> [1024, 8192]
        code_j = np.ascontiguousarray(
            code8[:, ksl].reshape(NB, GB, P, KS).transpose(0, 2, 1, 3)
        ).reshape(CODE_ROWS, CODE_COLS)
        cbt_j = np.ascontiguousarray(cb[ksl].T)  # [32, 512]
        in_maps.append({"code_s": code_j, "qo": qo_kc, "cbt": cbt_j})
    return in_maps


def run(quantized, code, codebook, trace=False, **spmd_kwargs):
    """Run the SPMD kernel; returns (loss_scalar, BassKernelResults)."""
    nc = _get_nc()
    in_maps = _make_in_maps(quantized, code, codebook)
    res = bass_utils.run_bass_kernel_spmd(
        nc, in_maps, core_ids=list(range(NCORES)), trace=trace, **spmd_kwargs
    )
    if COLLECTIVE == "host":
        parts = np.stack(
            [np.asarray(res.results[j]["loss"]).reshape(2) for j in range(NCORES)]
        )
        tot = parts.sum(axis=0, dtype=np.float32)
        loss = np.float32(tot[0] / max(tot[1], np.float32(C)))
    else:
        loss = np.float32(np.asarray(res.results[0]["loss"]).reshape(()))
    return np.asarray(loss, dtype=np.float32).reshape(()), res


def kernel(quantized, code, codebook):
    loss, _ = run(quantized, code, codebook)
    return loss
